# Initial kernel scaffold
#
"""LIF neuron with soft reset — Trainium2 Bass kernel, 8-way data parallel.

Problem: x (T=32, B=16, C=128, H=32, W=32) f32. Sequential scan over T:
    mem = 0.25*mem + x[t];  s[t] = (mem >= 1);  mem -= s[t]
Returns spikes (same shape, f32 values in {0,1}).

Sharding: batch dim B=16 split 2-per-core across 8 NeuronCores; the scan is
elementwise over (B,C,H,W) so cores are fully independent.

Per-core layout: the 2-batch slice of one timestep is 2*128*32*32 = 262144
contiguous floats -> one SBUF tile [128 partitions x 2048] f32 (1 MiB).

Per-step engine assignment (all ops bit-exact fp32; 0.25*m is exact since
0.25 is a power of two, so op-fusion differences cannot flip a spike):
  VectorE: m = (m*0.25)+x_t   (scalar_tensor_tensor, 1x)
           s = (m >= 1.0)     (tensor_scalar is_ge -> f32 {0,1}, 2x)
           m = m - s          (tensor_tensor, 1x)
  ScalarE: s_u8 = Copy(s)     (f32 {0,1} -> u8, offloads the convert)
  DMA out: u8 spikes (4x less HBM write traffic than f32; host upcasts).
"""

import numpy as np

T, B, C, H, W = 32, 16, 128, 32, 32
NCORES = 8
BPC = B // NCORES          # batches per core
P = 128                    # SBUF partitions
F = (BPC * C * H * W) // P # 2048 free-dim elements per step

_cache = {}


def _build():
    import concourse.bass as bass
    import concourse.mybir as mybir
    from concourse.tile import TileContext

    nc = bass.Bass()
    x_d = nc.dram_tensor("x", [T, P, F], mybir.dt.float32, kind="ExternalInput")
    o_d = nc.dram_tensor("o", [T, P, F], mybir.dt.uint8, kind="ExternalOutput")

    fp32 = mybir.dt.float32
    u8 = mybir.dt.uint8
    Alu = mybir.AluOpType
    Act = mybir.ActivationFunctionType

    with TileContext(nc) as tc:
        with (
            tc.tile_pool(name="mem", bufs=1) as mempool,
            tc.tile_pool(name="xin", bufs=4) as xpool,
            tc.tile_pool(name="spk", bufs=4) as spool,
            tc.tile_pool(name="out", bufs=4) as opool,
        ):
            m = mempool.tile([P, F], fp32, tag="m")
            nc.vector.memset(m, 0.0)
            for t in range(T):
                xt = xpool.tile([P, F], fp32, tag="x")
                nc.sync.dma_start(out=xt, in_=x_d[t])
                # m = 0.25*m + x_t   (0.25*m exact; single rounding on add)
                nc.vector.scalar_tensor_tensor(m, m, 0.25, xt, Alu.mult, Alu.add)
                s = spool.tile([P, F], fp32, tag="s")
                nc.vector.tensor_scalar(s, m, 1.0, None, Alu.is_ge)
                su = opool.tile([P, F], u8, tag="su")
                nc.scalar.activation(su, s, Act.Copy, bias=0.0, scale=1.0)
                nc.sync.dma_start(out=o_d[t], in_=su)
                # soft reset (m>=1 guarantees m-1 is the exact same rounding
                # the reference takes)
                nc.vector.tensor_tensor(m, m, s, Alu.subtract)
    return nc


def kernel(x: np.ndarray) -> np.ndarray:
    from concourse.bass_utils import run_bass_kernel_spmd

    assert x.shape == (T, B, C, H, W) and x.dtype == np.float32
    if "nc" not in _cache:
        _cache["nc"] = _build()
    nc = _cache["nc"]

    in_maps = []
    for k in range(NCORES):
        xk = np.ascontiguousarray(x[:, k * BPC : (k + 1) * BPC]).reshape(T, P, F)
        in_maps.append({"x": xk})

    res = run_bass_kernel_spmd(nc, in_maps, core_ids=list(range(NCORES)))
    _cache["last_result"] = res

    out = np.empty((T, B, C, H, W), dtype=np.float32)
    for k in range(NCORES):
        ok = res.results[k]["o"].reshape(T, BPC, C, H, W)
        out[:, k * BPC : (k + 1) * BPC] = ok.astype(np.float32)
    return out


# revision 6
# speedup vs baseline: 2.4894x; 2.4894x over previous
"""LIF neuron with soft reset — Trainium2 Bass kernel, 8-way data parallel.

Problem: x (T=32, B=16, C=128, H=32, W=32) f32. Sequential scan over T:
    mem = 0.25*mem + x[t];  s[t] = (mem >= 1);  mem -= s[t]
Returns spikes (same shape, f32 values in {0,1}).

Sharding: batch dim B=16 split 2-per-core across 8 NeuronCores; the scan is
elementwise over (B,C,H,W) so cores are fully independent.

Per-core layout: the 2-batch slice of one timestep is 2*128*32*32 = 262144
contiguous floats -> one SBUF tile [128 partitions x 2048] f32 (1 MiB).

Per-step engine assignment (all ops bit-exact fp32; 0.25*m is exact since
0.25 is a power of two, so op-fusion differences cannot flip a spike):
  VectorE: m = (m*0.25)+x_t   (scalar_tensor_tensor, 1x)
           s = (m >= 1.0)     (tensor_scalar is_ge -> f32 {0,1}, 2x)
           m = m - s          (tensor_tensor, 1x)
  ScalarE: s_u8 = Copy(s)     (f32 {0,1} -> u8, offloads the convert)
  DMA out: u8 spikes (4x less HBM write traffic than f32; host upcasts).
"""

import numpy as np

T, B, C, H, W = 32, 16, 128, 32, 32
NCORES = 8
BPC = B // NCORES          # batches per core
P = 128                    # SBUF partitions
F = (BPC * C * H * W) // P # 2048 free-dim elements per step

_cache = {}


def _build(reps: int = 1):
    import concourse.bacc as bacc
    import concourse.mybir as mybir
    from concourse.tile import TileContext

    nc = bacc.Bacc(None, target_bir_lowering=False)
    x_d = nc.dram_tensor("x", [T, P, F], mybir.dt.float32, kind="ExternalInput")
    o_d = nc.dram_tensor("o", [T, P, F], mybir.dt.uint8, kind="ExternalOutput")

    fp32 = mybir.dt.float32
    u8 = mybir.dt.uint8
    Alu = mybir.AluOpType
    Act = mybir.ActivationFunctionType

    with TileContext(nc) as tc:
        with (
            tc.tile_pool(name="mem", bufs=1) as mempool,
            tc.tile_pool(name="xin", bufs=4) as xpool,
            tc.tile_pool(name="spk", bufs=4) as spool,
            tc.tile_pool(name="out", bufs=4) as opool,
        ):
            m = mempool.tile([P, F], fp32, tag="m")
            for _ in range(reps):  # reps>1 only for benchmarking
                nc.vector.memset(m, 0.0)
                for t in range(T):
                    xt = xpool.tile([P, F], fp32, tag="x")
                    nc.sync.dma_start(out=xt, in_=x_d[t])
                    # m = 0.25*m + x_t (0.25*m exact; single rounding on add)
                    nc.vector.scalar_tensor_tensor(m, m, 0.25, xt, Alu.mult, Alu.add)
                    s = spool.tile([P, F], fp32, tag="s")
                    nc.vector.tensor_scalar(s, m, 1.0, None, Alu.is_ge)
                    su = opool.tile([P, F], u8, tag="su")
                    nc.scalar.activation(su, s, Act.Copy, bias=0.0, scale=1.0)
                    nc.sync.dma_start(out=o_d[t], in_=su)
                    # soft reset (m>=1 keeps m-1 at the reference's rounding)
                    nc.vector.tensor_tensor(m, m, s, Alu.subtract)
    nc.finalize()
    return nc


def kernel(x: np.ndarray) -> np.ndarray:
    from concourse.bass_utils import run_bass_kernel_spmd

    assert x.shape == (T, B, C, H, W) and x.dtype == np.float32
    if "nc" not in _cache:
        _cache["nc"] = _build()
    nc = _cache["nc"]

    in_maps = []
    for k in range(NCORES):
        xk = np.ascontiguousarray(x[:, k * BPC : (k + 1) * BPC]).reshape(T, P, F)
        in_maps.append({"x": xk})

    res = run_bass_kernel_spmd(nc, in_maps, core_ids=list(range(NCORES)))
    _cache["last_result"] = res

    out = np.empty((T, B, C, H, W), dtype=np.float32)
    for k in range(NCORES):
        ok = res.results[k]["o"].reshape(T, BPC, C, H, W)
        out[:, k * BPC : (k + 1) * BPC] = ok.astype(np.float32)
    return out
